# revision 10
# baseline (speedup 1.0000x reference)
"""Energy refinement kernel for Trainium2 (8 NeuronCores, SPMD row-sharded).

Math
----
reference() for L=4096 coords [L,3], contact_map [L,L]:
  e_bond  = mean((||c[i+1]-c[i]|| - 6)^2)                   O(L)     -> host
  d[i,j]  = ||c_i - c_j|| + 1e-8
  e_clash = sum_{j>=i+3} relu(3.4-d)^2 / L                  O(L^2)   -> device
  e_pair  = sum_{contact & |i-j|>=3} (d-9)^2 / max(n_c,1)   sparse   -> host
  total   = e_bond + 2*e_clash + 0.5*e_pair

Only e_clash is dense compute; e_pair touches the ~1% contact pairs, which a
host gather does exactly in f64 (shipping the 64MB contact map to the device
would cost ~22us of DMA per core alone, more than the whole kernel).

Device (folded symmetry, row-sharded over 8 cores):
  d is symmetric: 128-row block a only processes the 15-block cyclic column
  span [128(a+1), 128a+2048).  Each core owns 4 row blocks; their spans
  overlap, so one 18-block union of B columns ships per core (pre-gathered on
  host; SPMD program identical on all cores).

  d2 - n_i is computed by ONE K=12 bf16 matmul at 1 PE cycle/row:
    cross term -2*c_i.c_j to ~fp24 via hi/lo split (bf16 products are exact
    in the PE, fp32 accumulate): ch.b2h + cl.b2h + ch.b2l  (9 rows)
    n_j to ~fp24 via a 3-way bf16 split against an all-ones lhs (3 rows)
  |error| < 0.14 abs, zero-mean (validated vs f64 on the actual data).
  n_i (per-partition) rides the ACT bias input for free.

  Per row tile [128, 1920]:
    ACT : s = sqrt(d2 + 0.01) -> f16       (bias AP = n_i + 0.01)
    DVE : r = min(s,3.4) - 3.4   f16 4x    (r == 0 exactly for non-clash
                                            pairs; sqrt(neg) NaN from the
                                            rare near-duplicate pair flushes
                                            to 0 through the min)
    DVE : q = r*r                f16
    DVE : ts(q,0,0,add,add) accum -> acc   f16 row-sum per partition
  (gpsimd assists were tried and lose: pool tensor_scalar/stt fail walrus
   codegen, and a pool tensor_tensor slice adds cross-engine serialization
   that costs more than the DVE cycles it saves.)
Host finishing (f64): diag + sep-16 block pairs exactly (the unordered pairs
the span skips), minus the |i-j|<=2 band pairs the span wrongly includes,
plus exact e_bond and e_pair; one scalar combine.
"""

import numpy as np

L = 4096
NCORES = 8
RPC = L // NCORES          # 512 rows per core
RT = RPC // 128            # 4 row tiles of 128 partitions
SPAN = 15 * 128            # 1920 columns per row tile
UNION = 18 * 128           # 2304-column B union per core
K = 12
MIN_DIST = 3.4
TARGET_DIST = 9.0
IDEAL_BOND = 6.0
W_BOND, W_CLASH, W_PAIR = 1.0, 2.0, 0.5
SQRT_BIAS = 0.01


def _build_nc(reps=1, iters=1):
    """reps: python-unrolled passes; iters: hardware For_i loop around them
    (total passes = reps * iters; iters>1 is used only for benchmarking)."""
    import contextlib

    import concourse.bass as bass
    import concourse.bacc as bacc
    import concourse.mybir as mybir
    import concourse.tile as tile

    f32 = mybir.dt.float32
    f16 = mybir.dt.float16
    bf16 = mybir.dt.bfloat16
    AF = mybir.ActivationFunctionType
    ALU = mybir.AluOpType

    nc = bacc.Bacc(None)
    nab = RPC + UNION
    ab = nc.declare_dram_parameter("ab", [K, nab], bf16, isOutput=False)
    nbias = nc.declare_dram_parameter("nbias", [128, RT], f32, isOutput=False)
    o_c = nc.declare_dram_parameter("o_c", [128, RT], f32, isOutput=True)

    with tile.TileContext(nc) as tc:
        with (
            tc.tile_pool(name="const", bufs=1) as constp,
            tc.tile_pool(name="sp", bufs=2) as sp,
            tc.tile_pool(name="rp", bufs=2) as rp,
            tc.tile_pool(name="qp", bufs=2) as qp,
            tc.tile_pool(name="jp", bufs=2) as jpool,
            tc.tile_pool(name="psum", bufs=2, space=bass.MemorySpace.PSUM) as psum,
        ):
            ab_sb = constp.tile([K, nab], bf16)
            # split the load so tile 0's first matmul starts after chunk 1
            for lo, hi in ((0, RPC + 512), (RPC + 512, RPC + SPAN),
                           (RPC + SPAN, nab)):
                nc.sync.dma_start(ab_sb[:, lo:hi], ab[:, lo:hi])
            nb_sb = constp.tile([128, RT], f32)
            nc.sync.dma_start(nb_sb[:], nbias[:])
            acc = constp.tile([128, RT], f32)

            loop = tc.For_i(0, iters) if iters > 1 else contextlib.nullcontext()
            with loop:
                for _ in range(reps):
                    for it in range(RT):
                        rbase = RPC + it * 128
                        ps = psum.tile([128, SPAN], f32, tag="d2")
                        lhs = ab_sb[:, it * 128 : (it + 1) * 128]
                        off = 0
                        for n in (512, 512, 512, 384):
                            nc.tensor.matmul(
                                ps[:, off : off + n], lhs,
                                ab_sb[:, rbase + off : rbase + off + n],
                                start=True, stop=True,
                            )
                            off += n

                        s16 = sp.tile([128, SPAN], f16, tag="s16")
                        nc.scalar.activation(s16[:], ps[:], AF.Sqrt,
                                             bias=nb_sb[:, it : it + 1],
                                             scale=1.0)
                        r16 = rp.tile([128, SPAN], f16, tag="r16")
                        nc.vector.tensor_scalar(r16[:], s16[:], MIN_DIST,
                                                MIN_DIST, ALU.min, ALU.subtract)
                        q16 = qp.tile([128, SPAN], f16, tag="q16")
                        nc.vector.tensor_tensor(q16[:], r16[:], r16[:],
                                                ALU.mult)
                        jk = jpool.tile([128, SPAN], f16, tag="jk")
                        nc.vector.tensor_scalar(jk[:], q16[:], 0.0, 0.0,
                                                ALU.add, ALU.add,
                                                accum_out=acc[:, it : it + 1])

            nc.sync.dma_start(o_c[:], acc[:])
    nc.compile()
    return nc


def _host_inputs(coords):
    """Per-core K=12 bf16 operand block + per-row sqrt bias."""
    import ml_dtypes

    bf16 = ml_dtypes.bfloat16
    c = np.asarray(coords, dtype=np.float32)
    n2 = (c * c).sum(axis=1, dtype=np.float32).astype(np.float32)
    ch = c.astype(bf16)
    cl = (c - ch.astype(np.float32)).astype(bf16)
    b2 = (-2.0 * c).astype(np.float32)
    b2h = b2.astype(bf16)
    b2l = (b2 - b2h.astype(np.float32)).astype(bf16)
    njh = n2.astype(bf16)
    rem = n2 - njh.astype(np.float32)
    njm = rem.astype(bf16)
    njl = (rem - njm.astype(np.float32)).astype(bf16)
    ones = np.ones(L, dtype=bf16)

    # [12, L] A-side (lhsT source) and B-side (rhs source)
    Aside = np.stack([ch[:, 0], ch[:, 1], ch[:, 2],
                      cl[:, 0], cl[:, 1], cl[:, 2],
                      ch[:, 0], ch[:, 1], ch[:, 2],
                      ones, ones, ones])
    Bside = np.stack([b2h[:, 0], b2h[:, 1], b2h[:, 2],
                      b2h[:, 0], b2h[:, 1], b2h[:, 2],
                      b2l[:, 0], b2l[:, 1], b2l[:, 2],
                      njh, njm, njl])

    in_maps = []
    for r in range(NCORES):
        cols = np.arange(128 * (4 * r + 1), 128 * (4 * r + 1) + UNION) % L
        ab = np.concatenate([Aside[:, r * RPC : (r + 1) * RPC], Bside[:, cols]],
                            axis=1)
        nb = np.empty((128, RT), dtype=np.float32)
        for it in range(RT):
            i0 = (4 * r + it) * 128
            nb[:, it] = n2[i0 : i0 + 128] + np.float32(SQRT_BIAS)
        in_maps.append({"ab": np.ascontiguousarray(ab), "nbias": nb})
    return in_maps


def _host_block_clash(coords):
    """Exact f64 clash over diag and sep-16 block pairs (span-skipped)."""
    c = coords.astype(np.float64)
    iu, ju = np.triu_indices(128, k=1)
    m3 = (ju - iu) >= 3
    iu, ju = iu[m3], ju[m3]
    clash = 0.0
    for a in range(L // 128):
        i0 = a * 128
        blk = c[i0 : i0 + 128]
        dd = np.sqrt(((blk[iu] - blk[ju]) ** 2).sum(-1)) + 1e-8
        cl = np.maximum(MIN_DIST - dd, 0.0)
        clash += float((cl * cl).sum())
        if a < 16:
            blk2 = c[i0 + 2048 : i0 + 2176]
            d2 = np.sqrt(
                ((blk[:, None, :] - blk2[None, :, :]) ** 2).sum(-1)
            ) + 1e-8
            cl2 = np.maximum(MIN_DIST - d2, 0.0)
            clash += float((cl2 * cl2).sum())
    return clash


def _band_clash(coords):
    """f64 clash over the block-crossing |i-j|<=2 pairs the span includes."""
    c = coords.astype(np.float64)
    band = 0.0
    for off in (1, 2):
        i = np.arange(L - off)
        i = i[(i % 128) >= 128 - off]
        j = i + off
        d = np.sqrt(((c[i] - c[j]) ** 2).sum(-1)) + 1e-8
        cl = np.maximum(MIN_DIST - d, 0.0)
        band += float((cl * cl).sum())
    return band


_CACHE = {}


def kernel(coords, contact_map):
    from concourse.bass_utils import run_bass_kernel_spmd

    coords = np.asarray(coords, dtype=np.float32)
    contact_map = np.asarray(contact_map)
    in_maps = _host_inputs(coords)

    if "nc" not in _CACHE:
        _CACHE["nc"] = _build_nc()
    res = run_bass_kernel_spmd(_CACHE["nc"], in_maps, list(range(NCORES))).results

    S = 0.0
    for r in range(NCORES):
        S += float(res[r]["o_c"].astype(np.float64).sum())

    e_clash = (S - _band_clash(coords) + _host_block_clash(coords)) / L

    cd = coords.astype(np.float64)
    ii, jj = np.nonzero(contact_map > 0.5)
    n_pairs = max(len(ii), 1)
    sep_ok = np.abs(ii - jj) >= 3
    dp = np.sqrt(((cd[ii[sep_ok]] - cd[jj[sep_ok]]) ** 2).sum(-1)) + 1e-8
    e_pair = float(((dp - TARGET_DIST) ** 2).sum()) / n_pairs

    diff = cd[1:] - cd[:-1]
    bond = np.sqrt((diff * diff).sum(axis=1))
    e_bond = float(((bond - IDEAL_BOND) ** 2).mean())

    total = W_BOND * e_bond + W_CLASH * e_clash + W_PAIR * e_pair
    return np.array([total], dtype=np.float32)


# revision 13
# speedup vs baseline: 1.4642x; 1.4642x over previous
"""Energy refinement kernel for Trainium2 (8 NeuronCores, SPMD row-sharded).

Math
----
reference() for L=4096 coords [L,3], contact_map [L,L]:
  e_bond  = mean((||c[i+1]-c[i]|| - 6)^2)                   O(L)     -> host
  d[i,j]  = ||c_i - c_j|| + 1e-8
  e_clash = sum_{j>=i+3} relu(3.4-d)^2 / L                  O(L^2)   -> device
  e_pair  = sum_{contact & |i-j|>=3} (d-9)^2 / max(n_c,1)   sparse   -> host
  total   = e_bond + 2*e_clash + 0.5*e_pair

Only e_clash is dense compute; e_pair touches the ~1% contact pairs, which a
host gather does exactly in f64 (shipping the 64MB contact map to the device
would cost ~22us of DMA per core alone, more than the whole kernel).

Device (folded symmetry, row-sharded over 8 cores):
  d is symmetric: 128-row block a only processes the 15-block cyclic column
  span [128(a+1), 128a+2048).  Each core owns 4 row blocks; their spans
  overlap, so one 18-block union of B columns ships per core (pre-gathered on
  host; SPMD program identical on all cores).

  d2 - n_i is computed by ONE K=12 bf16 matmul at 1 PE cycle/row:
    cross term -2*c_i.c_j to ~fp24 via hi/lo split (bf16 products are exact
    in the PE, fp32 accumulate): ch.b2h + cl.b2h + ch.b2l  (9 rows)
    n_j to ~fp24 via a 3-way bf16 split against an all-ones lhs (3 rows)
  |error| < 0.14 abs, zero-mean (validated vs f64 on the actual data).
  n_i (per-partition) rides the ACT bias input for free.

  Per row tile [128, 1920]:
    ACT : s = sqrt(d2 + 0.01) -> f16       (bias AP = n_i + 0.01)
    DVE : r = min(s,3.4) - 3.4   f16       (r == 0 exactly for non-clash
                                            pairs; sqrt(neg) NaN from the
                                            rare near-duplicate pair flushes
                                            to 0 through the min)
    DVE : stt (1*r)*r accum -> acc         f16 square + row-sum in one op
  (measured on HW: the fused stt beats a tensor_tensor square + tensor_scalar
   reduce pair by ~29% end-to-end; gpsimd assists lose — pool tensor_scalar/
   stt fail walrus codegen and a pool tensor_tensor slice adds cross-engine
   serialization that costs more than the DVE cycles it saves; offloading a
   square slice to ACT Square+accum also loses — ACT is co-saturated.)
Host finishing (f64): diag + sep-16 block pairs exactly (the unordered pairs
the span skips), minus the |i-j|<=2 band pairs the span wrongly includes,
plus exact e_bond and e_pair; one scalar combine.
"""

import numpy as np

L = 4096
NCORES = 8
RPC = L // NCORES          # 512 rows per core
RT = RPC // 128            # 4 row tiles of 128 partitions
SPAN = 15 * 128            # 1920 columns per row tile
UNION = 18 * 128           # 2304-column B union per core
K = 12
MIN_DIST = 3.4
TARGET_DIST = 9.0
IDEAL_BOND = 6.0
W_BOND, W_CLASH, W_PAIR = 1.0, 2.0, 0.5
SQRT_BIAS = 0.01


def _build_nc(reps=1, iters=1):
    """reps: python-unrolled passes; iters: hardware For_i loop around them
    (total passes = reps * iters; iters>1 is used only for benchmarking)."""
    import contextlib

    import concourse.bass as bass
    import concourse.bacc as bacc
    import concourse.mybir as mybir
    import concourse.tile as tile

    f32 = mybir.dt.float32
    f16 = mybir.dt.float16
    bf16 = mybir.dt.bfloat16
    AF = mybir.ActivationFunctionType
    ALU = mybir.AluOpType

    nc = bacc.Bacc(None)
    nab = RPC + UNION
    ab = nc.declare_dram_parameter("ab", [K, nab], bf16, isOutput=False)
    nbias = nc.declare_dram_parameter("nbias", [128, RT], f32, isOutput=False)
    o_c = nc.declare_dram_parameter("o_c", [128, RT], f32, isOutput=True)

    with tile.TileContext(nc) as tc:
        with (
            tc.tile_pool(name="const", bufs=1) as constp,
            tc.tile_pool(name="sp", bufs=2) as sp,
            tc.tile_pool(name="rp", bufs=2) as rp,
            tc.tile_pool(name="jp", bufs=2) as jpool,
            tc.tile_pool(name="psum", bufs=2, space=bass.MemorySpace.PSUM) as psum,
        ):
            ab_sb = constp.tile([K, nab], bf16)
            # split the load so tile 0's first matmul starts after chunk 1
            for lo, hi in ((0, RPC + 512), (RPC + 512, RPC + SPAN),
                           (RPC + SPAN, nab)):
                nc.sync.dma_start(ab_sb[:, lo:hi], ab[:, lo:hi])
            nb_sb = constp.tile([128, RT], f32)
            nc.sync.dma_start(nb_sb[:], nbias[:])
            acc = constp.tile([128, RT], f32)

            loop = tc.For_i(0, iters) if iters > 1 else contextlib.nullcontext()
            with loop:
                for _ in range(reps):
                    for it in range(RT):
                        rbase = RPC + it * 128
                        ps = psum.tile([128, SPAN], f32, tag="d2")
                        lhs = ab_sb[:, it * 128 : (it + 1) * 128]
                        off = 0
                        for n in (512, 512, 512, 384):
                            nc.tensor.matmul(
                                ps[:, off : off + n], lhs,
                                ab_sb[:, rbase + off : rbase + off + n],
                                start=True, stop=True,
                            )
                            off += n

                        s16 = sp.tile([128, SPAN], f16, tag="s16")
                        nc.scalar.activation(s16[:], ps[:], AF.Sqrt,
                                             bias=nb_sb[:, it : it + 1],
                                             scale=1.0)
                        r16 = rp.tile([128, SPAN], f16, tag="r16")
                        nc.vector.tensor_scalar(r16[:], s16[:], MIN_DIST,
                                                MIN_DIST, ALU.min, ALU.subtract)
                        jk = jpool.tile([128, SPAN], f16, tag="jk")
                        nc.vector.scalar_tensor_tensor(
                            jk[:], r16[:], 1.0, r16[:], ALU.mult, ALU.mult,
                            accum_out=acc[:, it : it + 1])

            nc.sync.dma_start(o_c[:], acc[:])
    nc.compile()
    return nc


def _host_inputs(coords):
    """Per-core K=12 bf16 operand block + per-row sqrt bias."""
    import ml_dtypes

    bf16 = ml_dtypes.bfloat16
    c = np.asarray(coords, dtype=np.float32)
    n2 = (c * c).sum(axis=1, dtype=np.float32).astype(np.float32)
    ch = c.astype(bf16)
    cl = (c - ch.astype(np.float32)).astype(bf16)
    b2 = (-2.0 * c).astype(np.float32)
    b2h = b2.astype(bf16)
    b2l = (b2 - b2h.astype(np.float32)).astype(bf16)
    njh = n2.astype(bf16)
    rem = n2 - njh.astype(np.float32)
    njm = rem.astype(bf16)
    njl = (rem - njm.astype(np.float32)).astype(bf16)
    ones = np.ones(L, dtype=bf16)

    # [12, L] A-side (lhsT source) and B-side (rhs source)
    Aside = np.stack([ch[:, 0], ch[:, 1], ch[:, 2],
                      cl[:, 0], cl[:, 1], cl[:, 2],
                      ch[:, 0], ch[:, 1], ch[:, 2],
                      ones, ones, ones])
    Bside = np.stack([b2h[:, 0], b2h[:, 1], b2h[:, 2],
                      b2h[:, 0], b2h[:, 1], b2h[:, 2],
                      b2l[:, 0], b2l[:, 1], b2l[:, 2],
                      njh, njm, njl])

    in_maps = []
    for r in range(NCORES):
        cols = np.arange(128 * (4 * r + 1), 128 * (4 * r + 1) + UNION) % L
        ab = np.concatenate([Aside[:, r * RPC : (r + 1) * RPC], Bside[:, cols]],
                            axis=1)
        nb = np.empty((128, RT), dtype=np.float32)
        for it in range(RT):
            i0 = (4 * r + it) * 128
            nb[:, it] = n2[i0 : i0 + 128] + np.float32(SQRT_BIAS)
        in_maps.append({"ab": np.ascontiguousarray(ab), "nbias": nb})
    return in_maps


def _host_block_clash(coords):
    """Exact f64 clash over diag and sep-16 block pairs (span-skipped)."""
    c = coords.astype(np.float64)
    iu, ju = np.triu_indices(128, k=1)
    m3 = (ju - iu) >= 3
    iu, ju = iu[m3], ju[m3]
    clash = 0.0
    for a in range(L // 128):
        i0 = a * 128
        blk = c[i0 : i0 + 128]
        dd = np.sqrt(((blk[iu] - blk[ju]) ** 2).sum(-1)) + 1e-8
        cl = np.maximum(MIN_DIST - dd, 0.0)
        clash += float((cl * cl).sum())
        if a < 16:
            blk2 = c[i0 + 2048 : i0 + 2176]
            d2 = np.sqrt(
                ((blk[:, None, :] - blk2[None, :, :]) ** 2).sum(-1)
            ) + 1e-8
            cl2 = np.maximum(MIN_DIST - d2, 0.0)
            clash += float((cl2 * cl2).sum())
    return clash


def _band_clash(coords):
    """f64 clash over the block-crossing |i-j|<=2 pairs the span includes."""
    c = coords.astype(np.float64)
    band = 0.0
    for off in (1, 2):
        i = np.arange(L - off)
        i = i[(i % 128) >= 128 - off]
        j = i + off
        d = np.sqrt(((c[i] - c[j]) ** 2).sum(-1)) + 1e-8
        cl = np.maximum(MIN_DIST - d, 0.0)
        band += float((cl * cl).sum())
    return band


_CACHE = {}


def kernel(coords, contact_map):
    from concourse.bass_utils import run_bass_kernel_spmd

    coords = np.asarray(coords, dtype=np.float32)
    contact_map = np.asarray(contact_map)
    in_maps = _host_inputs(coords)

    if "nc" not in _CACHE:
        _CACHE["nc"] = _build_nc()
    res = run_bass_kernel_spmd(_CACHE["nc"], in_maps, list(range(NCORES))).results

    S = 0.0
    for r in range(NCORES):
        S += float(res[r]["o_c"].astype(np.float64).sum())

    e_clash = (S - _band_clash(coords) + _host_block_clash(coords)) / L

    cd = coords.astype(np.float64)
    ii, jj = np.nonzero(contact_map > 0.5)
    n_pairs = max(len(ii), 1)
    sep_ok = np.abs(ii - jj) >= 3
    dp = np.sqrt(((cd[ii[sep_ok]] - cd[jj[sep_ok]]) ** 2).sum(-1)) + 1e-8
    e_pair = float(((dp - TARGET_DIST) ** 2).sum()) / n_pairs

    diff = cd[1:] - cd[:-1]
    bond = np.sqrt((diff * diff).sum(axis=1))
    e_bond = float(((bond - IDEAL_BOND) ** 2).mean())

    total = W_BOND * e_bond + W_CLASH * e_clash + W_PAIR * e_pair
    return np.array([total], dtype=np.float32)
